# revision 15
# baseline (speedup 1.0000x reference)
"""Trainium2 Bass kernel for nn_CrossAttention_43258910605402.

Masked cross-attention, head-parallel over 8 NeuronCores (one head per core).

Math (per head h):
  q = x @ Wq[:, 64h:64h+64] * d^-0.5          [n=6912, 64]
  k = ctx @ Wk[:, 64h:64h+64]                 [m=3072, 64]
  v = ctx @ Wv[:, 64h:64h+64]                 [m=3072, 64]
  S = q @ k^T + mask                          [n, m],  mask = -80 * (m1_i & m2_j)
  A = exp(S)   (no row-max: |S| <= ~1 here; masked -> exp(S-80) ~ 1e-35 = 0)
  out_h^T = Wo^T @ (A @ v)^T                  unnormalized  [320, n]
  rsum_h = rowsum(A)                          [n]
Host: out = sum_h out_h^T / rsum_h, transpose, + bo (norm deferred to host).

Device schedule per (512-q-chunk, k-tile pair), software-pipelined so the
in-order PE instruction queue never waits on an exp in flight (a stalled AV
blocks later S matmuls and lets the HAM clock-gate re-throttle the PE array
to 1.2 GHz):

  pair j : S^T pair (2x bf16 matmul kaug.T @ qaug) -> s_ps_j
           exp_j: 3 of 4 pairs on ScalarE (exact exp -> bf16);
                  1 of 4 on VectorE via Schraudolph bf16 bits =
                  int16(S*2^7/ln2 + 16250.4)  (rel err <= 3.3% sawtooth,
                  systematic part cancels in the softmax ratio; the 25%
                  share keeps the added noise ~1% while unloading ScalarE)
           AV_{j-1}: bf16 matmul, 65-row vaug (row 64 = ones -> rowsum),
                  accumulated in PSUM.  The last AV of a chunk is flushed
                  after the NEXT chunk's first S pair, so chunk boundaries
                  don't stall the PE either.
  epilogue: oc <- oT (DVE), rsum row DMA, out^T-proj (wo stationary, oc
           moving, 3 strips), SBUF bounce, DMA out.

A ~5us burst of dummy bf16 matmuls at kernel start warms the HAM clock
gate (needs ~3.4us of sustained PE activity) while the first DMAs stream.

Inputs x^T / ctx^T are shipped as bf16 (halves DMA bytes; the matmuls run
bf16 anyway).  Mask sparsity: rows/cols host-permuted unmasked-first;
chunks fully in the masked-q tail run a shortened k loop (13 of 24 k-tiles
for this seed), with any masked-k spillover killed by the -80 mask column.
"""

import numpy as np

HEADS = 8
D = 64
DA = 65          # d + 1 mask/ones row
N = 6912         # query positions
M = 3072         # key positions
C = 320          # model dim
SCALE = D ** -0.5
NEG = -80.0      # masked logit offset; exp(-80) == 0 in fp32/bf16

LN2 = float(np.log(2.0))
A16 = float(2.0 ** 7) / LN2       # Schraudolph-to-bf16-bits constants
B16 = 16250.40
QCHUNK = 512
DVE_EXP_PERIOD = 3                # 1 of every 3 exp pairs runs on VectorE

_compiled = {}
_last_in_maps = None
_last_key = None


def _chunks(total, size):
    out = []
    o = 0
    while o < total:
        w = min(size, total - o)
        out.append((o, w))
        o += w
    return out


def _build_program(N=N, M=M, n0=None, m0=None):
    import concourse.bacc as bacc
    import concourse.tile as tile
    import concourse.mybir as mybir

    NKT = M // 128
    if n0 is None or m0 is None:
        n0, m0 = N, M
    NKT_SHORT = max(1, min(NKT, -(-m0 // 128)))
    f32 = mybir.dt.float32
    f32r = mybir.dt.float32r
    bf16 = mybir.dt.bfloat16
    i16 = mybir.dt.int16
    EXP = mybir.ActivationFunctionType.Exp
    MULT = mybir.AluOpType.mult
    ADD = mybir.AluOpType.add

    nc = bacc.Bacc("TRN2", target_bir_lowering=False, debug=False)

    xt_d = nc.dram_tensor("xt", [C, N], bf16, kind="ExternalInput").ap()
    ctxt_d = nc.dram_tensor("ctxt", [C, M], bf16, kind="ExternalInput").ap()
    # bf16 prep weights: wq(192) wkv(384) eye(64)
    wprep_d = nc.dram_tensor("wprep", [128, 640], bf16,
                             kind="ExternalInput").ap()
    wo_d = nc.dram_tensor("wo", [64, 320], f32, kind="ExternalInput").ap()
    m1_d = nc.dram_tensor("m1neg", [1, N], f32, kind="ExternalInput").ap()
    m2_d = nc.dram_tensor("m2col", [1, M], f32, kind="ExternalInput").ap()
    out_d = nc.dram_tensor("out", [C, N], f32, kind="ExternalOutput").ap()
    rsum_d = nc.dram_tensor("rsum", [1, N], f32, kind="ExternalOutput").ap()

    CCH = [(0, 128), (128, 128), (256, 64)]   # contraction tiles over C=320

    with tile.TileContext(nc) as tc:
        with (
            tc.tile_pool(name="persist", bufs=1) as persist,
            tc.tile_pool(name="stage", bufs=3) as stage,
            tc.tile_pool(name="qpool", bufs=2) as qpool,
            tc.tile_pool(name="attn", bufs=4) as apool,
            tc.tile_pool(name="oc", bufs=2) as ocpool,
            tc.tile_pool(name="outsb", bufs=3) as outsb,
        ):
            # ---- weights ------------------------------------------------
            wprep = persist.tile([128, 640], bf16, tag="wprep")
            nc.sync.dma_start(wprep[:], wprep_d[:])
            wo_st = stage.tile([64, 320], f32, tag="wost", bufs=1)
            nc.sync.dma_start(wo_st[:], wo_d[:])
            wo_r = persist.tile([64, 320], f32r, tag="wo_r")
            nc.vector.tensor_copy(wo_r[:], wo_st[:])
            wq_b = wprep[:, 0:192]
            wkv_b = wprep[:, 192:576]
            eye = wprep[0:64, 576:640]
            # PE warmup: ~5us of back-to-back dummy matmuls while the first
            # DMAs stream, so the HAM clock gate reaches 2.4 GHz before the
            # real pipeline starts (needs ~3.4us of sustained activity)
            warm_w = stage.tile([128, 512], bf16, tag="warmw", bufs=1)
            nc.vector.memset(warm_w[:], 0.0)

            def wqslice(i):
                c0, cw = CCH[i]
                return wq_b[0:cw, i * 64:(i + 1) * 64]

            def wkvslice(i):
                c0, cw = CCH[i]
                return wkv_b[0:cw, i * 128:(i + 1) * 128]

            # ---- ctx^T / x^T (bf16, host-transposed) ---------------------
            ct = [persist.tile([128, M], bf16, tag="ct0", name="ct0"),
                  persist.tile([128, M], bf16, tag="ct1", name="ct1"),
                  persist.tile([64, M], bf16, tag="ct2", name="ct2")]

            kaug = persist.tile([DA, M], bf16, tag="kaug")
            vt = persist.tile([64, M], bf16, tag="vt")
            vaug = persist.tile([128, NKT, DA], bf16, tag="vaug")
            ones_col = persist.tile([128, NKT, 1], f32, tag="ones_col")
            nc.vector.memset(ones_col[:], 1.0)
            nc.vector.tensor_copy(vaug[:, :, 64:65], ones_col[:])
            qaug = persist.tile([DA, N], bf16, tag="qaug")
            with (
                tc.tile_pool(name="sps", bufs=2, space="PSUM") as sps,
                tc.tile_pool(name="ops", bufs=2, space="PSUM") as ops,
                tc.tile_pool(name="mps", bufs=2, space="PSUM") as mps,
            ):
                warm_ps = mps.tile([128, 512], f32, tag="sm", name="warmps")
                for _w in range(14):
                    nc.tensor.matmul(warm_ps[:], warm_w[:, 0:128],
                                     warm_w[:], start=True, stop=True)

                kv_chunks = _chunks(M, 512)
                kv_next = [0]

                def emit_kv():
                    o, w = kv_chunks[kv_next[0]]
                    kv_next[0] += 1
                    for i, (c0, cw) in enumerate(CCH):
                        nc.gpsimd.dma_start(ct[i][0:cw, o:o + w],
                                            ctxt_d[c0:c0 + cw, o:o + w])
                    m2c = stage.tile([1, 512], f32, tag="m2c", bufs=2)
                    nc.sync.dma_start(m2c[0:1, 0:w], m2_d[:, o:o + w])
                    nc.vector.tensor_copy(kaug[64:65, o:o + w], m2c[0:1, 0:w])
                    # k rows 0:64 and v rows 64:128 share the moving operand
                    kvps = mps.tile([128, 512], f32, tag="sm", name="kvps")
                    for i in range(3):
                        nc.tensor.matmul(kvps[:, 0:w], wkvslice(i),
                                         ct[i][0:CCH[i][1], o:o + w],
                                         start=(i == 0), stop=(i == 2))
                    nc.vector.tensor_copy(kaug[0:64, o:o + w],
                                          kvps[0:64, 0:w])
                    nc.vector.tensor_copy(vt[:, o:o + w], kvps[64:128, 0:w])
                    for j in range(o // 128, min(NKT, (o + w) // 128)):
                        vp = mps.tile([128, 64], bf16, tag="sm", name="vp")
                        nc.tensor.transpose(vp[:], vt[:, j * 128:(j + 1) * 128],
                                            eye[:])
                        nc.vector.tensor_copy(vaug[:, j, 0:64], vp[:])

                qprep_chunks = _chunks(N, 512)
                qprep_next = [0]

                def emit_qprep():
                    qo, qw = qprep_chunks[qprep_next[0]]
                    qprep_next[0] += 1
                    xt = [qpool.tile([128, 512], bf16, tag="xt0", name="xt0"),
                          qpool.tile([128, 512], bf16, tag="xt1", name="xt1"),
                          qpool.tile([64, 512], bf16, tag="xt2", name="xt2")]
                    for i, (c0, cw) in enumerate(CCH):
                        nc.gpsimd.dma_start(xt[i][0:cw, 0:qw],
                                            xt_d[c0:c0 + cw, qo:qo + qw])
                    m1c = stage.tile([1, 512], f32, tag="m1c", bufs=2)
                    nc.sync.dma_start(m1c[0:1, 0:qw], m1_d[:, qo:qo + qw])
                    nc.vector.tensor_copy(qaug[64:65, qo:qo + qw],
                                          m1c[0:1, 0:qw])
                    qp = mps.tile([64, 512], f32, tag="sm", name="qp")
                    for i in range(3):
                        nc.tensor.matmul(qp[0:64, 0:qw], wqslice(i),
                                         xt[i][0:CCH[i][1], 0:qw],
                                         start=(i == 0), stop=(i == 2))
                    nc.vector.tensor_copy(qaug[0:64, qo:qo + qw], qp[0:64, 0:qw])

                # all ctx DMAs + k/v prep up front: ct tiles are persistent
                # and the dense matmul stream keeps the HAM clock gate warm
                while kv_next[0] < len(kv_chunks):
                    emit_kv()

                pending_epi = [None]
                n0r = min(N, -(-n0 // 128) * 128)
                chunk_list = _chunks(n0r, QCHUNK) + [
                    (n0r + o, w) for (o, w) in _chunks(N - n0r, QCHUNK)]
                kv_next[0] = 0
                qprep_next[0] = 0
                exp_tick = [0]
                # pending AV work, delayed so the PE queue never waits on an
                # exp in flight; carried across chunk boundaries
                pend = [None]   # (at, jj0, ntiles, oT, qw, final_or_None)

                def flush_pend():
                    at, jj0, nt, p_oT, p_qw, final = pend[0]
                    pend[0] = None
                    for p in range(nt):
                        nc.tensor.matmul(
                            p_oT[0:65, 0:p_qw], vaug[:, jj0 + p, :],
                            at[:, p * 512:p * 512 + p_qw],
                            start=(jj0 + p == 0),
                            stop=final is not None and p == nt - 1,
                            skip_group_check=True)
                    if final is not None:
                        final()

                for (qo, qw) in chunk_list:
                    # keep q-prep one main-chunk ahead of consumption
                    target = min(N, qo + qw + QCHUNK)
                    while (qprep_next[0] < len(qprep_chunks)
                           and qprep_chunks[qprep_next[0]][0] < target):
                        emit_qprep()
                    nkt_c = NKT_SHORT if qo >= n0r else NKT

                    oT = ops.tile([DA, QCHUNK], f32, tag="oT")

                    def chunk_drain(oT=oT, qo=qo, qw=qw):
                        # runs right after the chunk's last AV matmul
                        oc = ocpool.tile([DA, QCHUNK], f32r, tag="oc")
                        nc.vector.tensor_copy(oc[:, 0:qw], oT[:, 0:qw])
                        nc.sync.dma_start(rsum_d[0:1, qo:qo + qw],
                                          oc[64:65, 0:qw].bitcast(f32))

                        def epilogue():
                            # out^T strips: stationary wo slice, moving oc
                            for (s0, sw) in [(0, 128), (128, 128), (256, 64)]:
                                pps2 = mps.tile([128, 512], f32, tag="sm",
                                                name="pps2")
                                nc.tensor.matmul(pps2[0:sw, 0:qw],
                                                 wo_r[:, s0:s0 + sw],
                                                 oc[0:64, 0:qw],
                                                 start=True, stop=True)
                                ot_sb = outsb.tile([128, 512], f32,
                                                   tag="osb")
                                nc.vector.tensor_copy(ot_sb[0:sw, 0:qw],
                                                      pps2[0:sw, 0:qw])
                                nc.sync.dma_start(
                                    out_d[s0:s0 + sw, qo:qo + qw],
                                    ot_sb[0:sw, 0:qw])

                        pending_epi[0] = epilogue

                    jj = 0
                    while jj < nkt_c:
                        pair = min(2, nkt_c - jj)
                        s_ps = sps.tile([128, 1024], f32, tag="s")
                        for p in range(pair):
                            nc.tensor.matmul(
                                s_ps[:, p * 512:p * 512 + qw],
                                kaug[:, (jj + p) * 128:(jj + p + 1) * 128],
                                qaug[:, qo:qo + qw],
                                start=True, stop=True)
                        at = apool.tile([128, 1024], bf16, tag="attn")
                        on_dve = exp_tick[0] % DVE_EXP_PERIOD == (
                            DVE_EXP_PERIOD - 1)
                        exp_tick[0] += 1
                        if pair == 2 and qw == 512:
                            if on_dve:
                                nc.vector.tensor_scalar(
                                    at[:].bitcast(i16), s_ps[:, 0:1024],
                                    A16, B16, MULT, ADD)
                            else:
                                nc.scalar.activation(at[:], s_ps[:, 0:1024],
                                                     EXP)
                        else:
                            for p in range(pair):
                                if on_dve:
                                    nc.vector.tensor_scalar(
                                        at[:, p * 512:p * 512 + qw]
                                        .bitcast(i16),
                                        s_ps[:, p * 512:p * 512 + qw],
                                        A16, B16, MULT, ADD)
                                else:
                                    nc.scalar.activation(
                                        at[:, p * 512:p * 512 + qw],
                                        s_ps[:, p * 512:p * 512 + qw], EXP)
                        if pend[0] is not None:
                            flush_pend()
                        last_pair = jj + pair >= nkt_c
                        if last_pair:
                            # fire the previous chunk's projection here so
                            # it sits between real attention matmuls
                            if pending_epi[0] is not None:
                                pending_epi[0]()
                                pending_epi[0] = None
                            pend[0] = (at, jj, pair, oT, qw, chunk_drain)
                        else:
                            pend[0] = (at, jj, pair, oT, qw, None)
                        jj += pair

                if pend[0] is not None:
                    flush_pend()
                if pending_epi[0] is not None:
                    pending_epi[0]()
                    pending_epi[0] = None

    nc.compile()
    return nc


def _get_compiled(n0=None, m0=None):
    key = (n0, m0)
    if key not in _compiled:
        _compiled[key] = _build_program(n0=n0, m0=m0)
    return _compiled[key]


def kernel(x, context, mask1, mask2, Wq, Wk, Wv, Wo, bo):
    import ml_dtypes
    from concourse import bass_utils

    global _last_in_maps, _last_key

    bf16 = ml_dtypes.bfloat16
    x = np.asarray(x, dtype=np.float32)
    context = np.asarray(context, dtype=np.float32)
    mask1 = np.asarray(mask1, dtype=np.float32)
    mask2 = np.asarray(mask2, dtype=np.float32)
    Wq = np.asarray(Wq, dtype=np.float32)
    Wk = np.asarray(Wk, dtype=np.float32)
    Wv = np.asarray(Wv, dtype=np.float32)
    Wo = np.asarray(Wo, dtype=np.float32)
    bo = np.asarray(bo, dtype=np.float32)

    b = x.shape[0]
    assert b == 1 and x.shape[1] == N and context.shape[1] == M

    # nearest-resize masks exactly as the reference does
    dxq = int((N // 12) ** 0.5)
    mH, mW = 4 * dxq, 3 * dxq
    dxk = int((M // 12) ** 0.5)
    mh, mw = 4 * dxk, 3 * dxk
    Hm, Wm = mask1.shape[-2], mask1.shape[-1]
    m1 = mask1[0, 0][(np.arange(mH) * Hm) // mH][:, (np.arange(mW) * Wm) // mW] >= 0.5
    m2 = mask2[0, 0][(np.arange(mh) * Hm) // mh][:, (np.arange(mw) * Wm) // mw] >= 0.5

    m1f = m1.reshape(-1)
    m2f = m2.reshape(-1)

    # group unmasked rows/cols first so masked-q chunks can use a short k loop
    qperm = np.argsort(m1f, kind="stable")       # False (unmasked) first
    kperm = np.argsort(m2f, kind="stable")
    n0 = int((~m1f).sum())
    m0 = int((~m2f).sum())
    use_sparse = n0 < N and m0 >= 128
    if not use_sparse:
        qperm = np.arange(N)
        kperm = np.arange(M)
        n0s, m0s = None, None
    else:
        n0s, m0s = n0, m0

    m1neg = np.where(m1f[qperm], np.float32(NEG), np.float32(0.0))
    m2col = m2f[kperm].astype(np.float32)
    xT = np.ascontiguousarray(x[0].T[:, qperm]).astype(bf16)
    ctxT = np.ascontiguousarray(context[0].T[:, kperm]).astype(bf16)

    def pack3(w):
        # [320, 64] -> [128, 192] (c-tiles of 128/128/64 side by side)
        p = np.zeros((128, 192), np.float32)
        p[:, 0:64] = w[0:128]
        p[:, 64:128] = w[128:256]
        p[0:64, 128:192] = w[256:320]
        return p

    def pack3kv(wk, wv):
        # [320, 64]x2 -> [128, 384]: per c-tile [cw, 128] = [k | v]
        p = np.zeros((128, 384), np.float32)
        for i, (c0, cw) in enumerate([(0, 128), (128, 128), (256, 64)]):
            p[0:cw, i * 128:i * 128 + 64] = wk[c0:c0 + cw]
            p[0:cw, i * 128 + 64:(i + 1) * 128] = wv[c0:c0 + cw]
        return p

    def wprep_pack(h):
        p = np.zeros((128, 640), np.float32)
        p[:, 0:192] = pack3(Wq[:, h * D:(h + 1) * D] * np.float32(SCALE))
        p[:, 192:576] = pack3kv(Wk[:, h * D:(h + 1) * D],
                                Wv[:, h * D:(h + 1) * D])
        p[0:64, 576:640] = np.eye(64, dtype=np.float32)
        return p.astype(bf16)

    in_maps = []
    for h in range(HEADS):
        in_maps.append({
            "xt": xT,
            "ctxt": ctxT,
            "wprep": wprep_pack(h),
            "wo": np.ascontiguousarray(Wo[h * D:(h + 1) * D, :]),
            "m1neg": m1neg.reshape(1, N),
            "m2col": m2col.reshape(1, M),
        })
    _last_in_maps = in_maps
    _last_key = (n0s, m0s)

    nc = _get_compiled(n0s, m0s)
    res = bass_utils.run_bass_kernel_spmd(nc, in_maps, list(range(HEADS)))
    outT = np.zeros((C, N), dtype=np.float32)
    for h in range(HEADS):
        r = res.results[h]
        outT += r["out"] / r["rsum"].reshape(1, N)
    out = outT.T + bo
    inv = np.empty(N, dtype=np.int64)
    inv[qperm] = np.arange(N)
    out = out[inv]
    return np.ascontiguousarray(out).reshape(1, N, C)


# revision 16
# speedup vs baseline: 1.2345x; 1.2345x over previous
"""Trainium2 Bass kernel for nn_CrossAttention_43258910605402.

Masked cross-attention, head-parallel over 8 NeuronCores (one head per core).

Math (per head h):
  q = x @ Wq[:, 64h:64h+64] * d^-0.5          [n=6912, 64]
  k = ctx @ Wk[:, 64h:64h+64]                 [m=3072, 64]
  v = ctx @ Wv[:, 64h:64h+64]                 [m=3072, 64]
  S = q @ k^T + mask                          [n, m],  mask = -80 * (m1_i & m2_j)
  A = exp(S)   (no row-max: |S| <= ~1 here; masked -> exp(S-80) ~ 1e-35 = 0)
  out_h^T = Wo^T @ (A @ v)^T                  unnormalized  [320, n]
  rsum_h = rowsum(A)                          [n]
Host: out = sum_h out_h^T / rsum_h, transpose, + bo (norm deferred to host).

Device schedule per (512-q-chunk, k-tile pair), software-pipelined so the
in-order PE instruction queue never waits on an exp in flight (a stalled AV
blocks later S matmuls and lets the HAM clock-gate re-throttle the PE array
to 1.2 GHz):

  pair j : S^T pair (2x bf16 matmul kaug.T @ qaug) -> s_ps_j
           exp_j: 3 of 4 pairs on ScalarE (exact exp -> bf16);
                  1 of 4 on VectorE via Schraudolph bf16 bits =
                  int16(S*2^7/ln2 + 16250.4)  (rel err <= 3.3% sawtooth,
                  systematic part cancels in the softmax ratio; the 25%
                  share keeps the added noise ~1% while unloading ScalarE)
           AV_{j-1}: bf16 matmul, 65-row vaug (row 64 = ones -> rowsum),
                  accumulated in PSUM.  The last AV of a chunk is flushed
                  after the NEXT chunk's first S pair, so chunk boundaries
                  don't stall the PE either.
  epilogue: oc <- oT (DVE), rsum row DMA, out^T-proj (wo stationary, oc
           moving, 3 strips), SBUF bounce, DMA out.

A ~5us burst of dummy bf16 matmuls at kernel start warms the HAM clock
gate (needs ~3.4us of sustained PE activity) while the first DMAs stream.

Inputs x^T / ctx^T are shipped as bf16 (halves DMA bytes; the matmuls run
bf16 anyway).  Mask sparsity: rows/cols host-permuted unmasked-first;
chunks fully in the masked-q tail run a shortened k loop (13 of 24 k-tiles
for this seed), with any masked-k spillover killed by the -80 mask column.
"""

import numpy as np

HEADS = 8
D = 64
DA = 65          # d + 1 mask/ones row
N = 6912         # query positions
M = 3072         # key positions
C = 320          # model dim
SCALE = D ** -0.5
NEG = -80.0      # masked logit offset; exp(-80) == 0 in fp32/bf16

LN2 = float(np.log(2.0))
A16 = float(2.0 ** 7) / LN2       # Schraudolph-to-bf16-bits constants
B16 = 16250.40
QCHUNK = 512
DVE_EXP_PERIOD = 3                # 1 of every 3 exp pairs runs on VectorE

_compiled = {}
_last_in_maps = None
_last_key = None


def _chunks(total, size):
    out = []
    o = 0
    while o < total:
        w = min(size, total - o)
        out.append((o, w))
        o += w
    return out


def _build_program(N=N, M=M, n0=None, m0=None):
    import concourse.bacc as bacc
    import concourse.tile as tile
    import concourse.mybir as mybir

    NKT = M // 128
    if n0 is None or m0 is None:
        n0, m0 = N, M
    NKT_SHORT = max(1, min(NKT, -(-m0 // 128)))
    f32 = mybir.dt.float32
    f32r = mybir.dt.float32r
    bf16 = mybir.dt.bfloat16
    i16 = mybir.dt.int16
    EXP = mybir.ActivationFunctionType.Exp
    MULT = mybir.AluOpType.mult
    ADD = mybir.AluOpType.add

    nc = bacc.Bacc("TRN2", target_bir_lowering=False, debug=False)

    xt_d = nc.dram_tensor("xt", [C, N], bf16, kind="ExternalInput").ap()
    ctxt_d = nc.dram_tensor("ctxt", [C, M], bf16, kind="ExternalInput").ap()
    # bf16 prep weights: wq(192) wkv(384) eye(64)
    wprep_d = nc.dram_tensor("wprep", [128, 640], bf16,
                             kind="ExternalInput").ap()
    wo_d = nc.dram_tensor("wo", [64, 320], f32, kind="ExternalInput").ap()
    m1_d = nc.dram_tensor("m1neg", [1, N], f32, kind="ExternalInput").ap()
    m2_d = nc.dram_tensor("m2col", [1, M], f32, kind="ExternalInput").ap()
    out_d = nc.dram_tensor("out", [C, N], f32, kind="ExternalOutput").ap()
    rsum_d = nc.dram_tensor("rsum", [1, N], f32, kind="ExternalOutput").ap()

    CCH = [(0, 128), (128, 128), (256, 64)]   # contraction tiles over C=320

    with tile.TileContext(nc) as tc:
        with (
            tc.tile_pool(name="persist", bufs=1) as persist,
            tc.tile_pool(name="stage", bufs=3) as stage,
            tc.tile_pool(name="qpool", bufs=2) as qpool,
            tc.tile_pool(name="attn", bufs=4) as apool,
            tc.tile_pool(name="oc", bufs=2) as ocpool,
            tc.tile_pool(name="outsb", bufs=3) as outsb,
        ):
            # ---- weights ------------------------------------------------
            wprep = persist.tile([128, 640], bf16, tag="wprep")
            nc.sync.dma_start(wprep[:], wprep_d[:])
            wo_st = stage.tile([64, 320], f32, tag="wost", bufs=1)
            nc.sync.dma_start(wo_st[:], wo_d[:])
            wo_r = persist.tile([64, 320], f32r, tag="wo_r")
            nc.vector.tensor_copy(wo_r[:], wo_st[:])
            wq_b = wprep[:, 0:192]
            wkv_b = wprep[:, 192:576]
            eye = wprep[0:64, 576:640]
            # PE warmup: ~5us of back-to-back dummy matmuls while the first
            # DMAs stream, so the HAM clock gate reaches 2.4 GHz before the
            # real pipeline starts (needs ~3.4us of sustained activity)
            warm_w = stage.tile([128, 512], bf16, tag="warmw", bufs=1)
            nc.vector.memset(warm_w[:], 0.0)

            def wqslice(i):
                c0, cw = CCH[i]
                return wq_b[0:cw, i * 64:(i + 1) * 64]

            def wkvslice(i):
                c0, cw = CCH[i]
                return wkv_b[0:cw, i * 128:(i + 1) * 128]

            # ---- ctx^T / x^T (bf16, host-transposed) ---------------------
            ct = [persist.tile([128, M], bf16, tag="ct0", name="ct0"),
                  persist.tile([128, M], bf16, tag="ct1", name="ct1"),
                  persist.tile([64, M], bf16, tag="ct2", name="ct2")]

            kaug = persist.tile([DA, M], bf16, tag="kaug")
            vt = persist.tile([64, M], bf16, tag="vt")
            vaug = persist.tile([128, NKT, DA], bf16, tag="vaug")
            ones_col = persist.tile([128, NKT, 1], f32, tag="ones_col")
            nc.vector.memset(ones_col[:], 1.0)
            nc.vector.tensor_copy(vaug[:, :, 64:65], ones_col[:])
            qaug = persist.tile([DA, N], bf16, tag="qaug")
            with (
                tc.tile_pool(name="sps", bufs=2, space="PSUM") as sps,
                tc.tile_pool(name="ops", bufs=2, space="PSUM") as ops,
                tc.tile_pool(name="mps", bufs=2, space="PSUM") as mps,
            ):
                warm_ps = mps.tile([128, 512], f32, tag="sm", name="warmps")
                for _w in range(14):
                    nc.tensor.matmul(warm_ps[:], warm_w[:, 0:128],
                                     warm_w[:], start=True, stop=True)

                kv_chunks = _chunks(M, 512)
                kv_next = [0]

                def emit_kv():
                    o, w = kv_chunks[kv_next[0]]
                    kv_next[0] += 1
                    for i, (c0, cw) in enumerate(CCH):
                        nc.gpsimd.dma_start(ct[i][0:cw, o:o + w],
                                            ctxt_d[c0:c0 + cw, o:o + w])
                    m2c = stage.tile([1, 512], f32, tag="m2c", bufs=2)
                    nc.sync.dma_start(m2c[0:1, 0:w], m2_d[:, o:o + w])
                    nc.vector.tensor_copy(kaug[64:65, o:o + w], m2c[0:1, 0:w])
                    # k rows 0:64 and v rows 64:128 share the moving operand
                    kvps = mps.tile([128, 512], f32, tag="sm", name="kvps")
                    for i in range(3):
                        nc.tensor.matmul(kvps[:, 0:w], wkvslice(i),
                                         ct[i][0:CCH[i][1], o:o + w],
                                         start=(i == 0), stop=(i == 2))
                    nc.vector.tensor_copy(kaug[0:64, o:o + w],
                                          kvps[0:64, 0:w])
                    nc.vector.tensor_copy(vt[:, o:o + w], kvps[64:128, 0:w])
                    for j in range(o // 128, min(NKT, (o + w) // 128)):
                        vp = mps.tile([128, 64], bf16, tag="sm", name="vp")
                        nc.tensor.transpose(vp[:], vt[:, j * 128:(j + 1) * 128],
                                            eye[:])
                        nc.vector.tensor_copy(vaug[:, j, 0:64], vp[:])

                qprep_chunks = _chunks(N, 512)
                qprep_next = [0]

                def emit_qprep():
                    qo, qw = qprep_chunks[qprep_next[0]]
                    qprep_next[0] += 1
                    xt = [qpool.tile([128, 512], bf16, tag="xt0", name="xt0"),
                          qpool.tile([128, 512], bf16, tag="xt1", name="xt1"),
                          qpool.tile([64, 512], bf16, tag="xt2", name="xt2")]
                    for i, (c0, cw) in enumerate(CCH):
                        nc.gpsimd.dma_start(xt[i][0:cw, 0:qw],
                                            xt_d[c0:c0 + cw, qo:qo + qw])
                    m1c = stage.tile([1, 512], f32, tag="m1c", bufs=2)
                    nc.sync.dma_start(m1c[0:1, 0:qw], m1_d[:, qo:qo + qw])
                    nc.vector.tensor_copy(qaug[64:65, qo:qo + qw],
                                          m1c[0:1, 0:qw])
                    qp = mps.tile([64, 512], f32, tag="sm", name="qp")
                    for i in range(3):
                        nc.tensor.matmul(qp[0:64, 0:qw], wqslice(i),
                                         xt[i][0:CCH[i][1], 0:qw],
                                         start=(i == 0), stop=(i == 2))
                    nc.vector.tensor_copy(qaug[0:64, qo:qo + qw], qp[0:64, 0:qw])

                pending_epi = [None]
                n0r = min(N, -(-n0 // 128) * 128)
                chunk_list = _chunks(n0r, QCHUNK) + [
                    (n0r + o, w) for (o, w) in _chunks(N - n0r, QCHUNK)]
                kv_next[0] = 0
                qprep_next[0] = 0
                exp_tick = [0]
                # pending AV work, delayed so the PE queue never waits on an
                # exp in flight; carried across chunk boundaries
                pend = [None]   # (at, jj0, ntiles, oT, qw, final_or_None)

                def flush_pend():
                    at, jj0, nt, p_oT, p_qw, final = pend[0]
                    pend[0] = None
                    for p in range(nt):
                        nc.tensor.matmul(
                            p_oT[0:65, 0:p_qw], vaug[:, jj0 + p, :],
                            at[:, p * 512:p * 512 + p_qw],
                            start=(jj0 + p == 0),
                            stop=final is not None and p == nt - 1,
                            skip_group_check=True)
                    if final is not None:
                        final()

                for (qo, qw) in chunk_list:
                    # keep q-prep one main-chunk ahead of consumption
                    target = min(N, qo + qw + QCHUNK)
                    while (qprep_next[0] < len(qprep_chunks)
                           and qprep_chunks[qprep_next[0]][0] < target):
                        emit_qprep()
                    nkt_c = NKT_SHORT if qo >= n0r else NKT

                    oT = ops.tile([DA, QCHUNK], f32, tag="oT")

                    def chunk_drain(oT=oT, qo=qo, qw=qw):
                        # runs right after the chunk's last AV matmul
                        oc = ocpool.tile([DA, QCHUNK], f32r, tag="oc")
                        nc.vector.tensor_copy(oc[:, 0:qw], oT[:, 0:qw])
                        nc.sync.dma_start(rsum_d[0:1, qo:qo + qw],
                                          oc[64:65, 0:qw].bitcast(f32))

                        def epilogue():
                            # out^T strips: stationary wo slice, moving oc
                            for (s0, sw) in [(0, 128), (128, 128), (256, 64)]:
                                pps2 = mps.tile([128, 512], f32, tag="sm",
                                                name="pps2")
                                nc.tensor.matmul(pps2[0:sw, 0:qw],
                                                 wo_r[:, s0:s0 + sw],
                                                 oc[0:64, 0:qw],
                                                 start=True, stop=True)
                                ot_sb = outsb.tile([128, 512], f32,
                                                   tag="osb")
                                nc.vector.tensor_copy(ot_sb[0:sw, 0:qw],
                                                      pps2[0:sw, 0:qw])
                                nc.sync.dma_start(
                                    out_d[s0:s0 + sw, qo:qo + qw],
                                    ot_sb[0:sw, 0:qw])

                        pending_epi[0] = epilogue

                    jj = 0
                    while jj < nkt_c:
                        while (kv_next[0] < len(kv_chunks)
                               and kv_next[0] * 4 < min(nkt_c, jj + 8)):
                            emit_kv()
                        pair = min(2, nkt_c - jj)
                        s_ps = sps.tile([128, 1024], f32, tag="s")
                        for p in range(pair):
                            nc.tensor.matmul(
                                s_ps[:, p * 512:p * 512 + qw],
                                kaug[:, (jj + p) * 128:(jj + p + 1) * 128],
                                qaug[:, qo:qo + qw],
                                start=True, stop=True)
                        at = apool.tile([128, 1024], bf16, tag="attn")
                        on_dve = exp_tick[0] % DVE_EXP_PERIOD == (
                            DVE_EXP_PERIOD - 1)
                        exp_tick[0] += 1
                        if pair == 2 and qw == 512:
                            if on_dve:
                                nc.vector.tensor_scalar(
                                    at[:].bitcast(i16), s_ps[:, 0:1024],
                                    A16, B16, MULT, ADD)
                            else:
                                nc.scalar.activation(at[:], s_ps[:, 0:1024],
                                                     EXP)
                        else:
                            for p in range(pair):
                                if on_dve:
                                    nc.vector.tensor_scalar(
                                        at[:, p * 512:p * 512 + qw]
                                        .bitcast(i16),
                                        s_ps[:, p * 512:p * 512 + qw],
                                        A16, B16, MULT, ADD)
                                else:
                                    nc.scalar.activation(
                                        at[:, p * 512:p * 512 + qw],
                                        s_ps[:, p * 512:p * 512 + qw], EXP)
                        if pend[0] is not None:
                            flush_pend()
                        last_pair = jj + pair >= nkt_c
                        if last_pair:
                            # fire the previous chunk's projection here so
                            # it sits between real attention matmuls
                            if pending_epi[0] is not None:
                                pending_epi[0]()
                                pending_epi[0] = None
                            pend[0] = (at, jj, pair, oT, qw, chunk_drain)
                        else:
                            pend[0] = (at, jj, pair, oT, qw, None)
                        jj += pair

                if pend[0] is not None:
                    flush_pend()
                if pending_epi[0] is not None:
                    pending_epi[0]()
                    pending_epi[0] = None

    nc.compile()
    return nc


def _get_compiled(n0=None, m0=None):
    key = (n0, m0)
    if key not in _compiled:
        _compiled[key] = _build_program(n0=n0, m0=m0)
    return _compiled[key]


def kernel(x, context, mask1, mask2, Wq, Wk, Wv, Wo, bo):
    import ml_dtypes
    from concourse import bass_utils

    global _last_in_maps, _last_key

    bf16 = ml_dtypes.bfloat16
    x = np.asarray(x, dtype=np.float32)
    context = np.asarray(context, dtype=np.float32)
    mask1 = np.asarray(mask1, dtype=np.float32)
    mask2 = np.asarray(mask2, dtype=np.float32)
    Wq = np.asarray(Wq, dtype=np.float32)
    Wk = np.asarray(Wk, dtype=np.float32)
    Wv = np.asarray(Wv, dtype=np.float32)
    Wo = np.asarray(Wo, dtype=np.float32)
    bo = np.asarray(bo, dtype=np.float32)

    b = x.shape[0]
    assert b == 1 and x.shape[1] == N and context.shape[1] == M

    # nearest-resize masks exactly as the reference does
    dxq = int((N // 12) ** 0.5)
    mH, mW = 4 * dxq, 3 * dxq
    dxk = int((M // 12) ** 0.5)
    mh, mw = 4 * dxk, 3 * dxk
    Hm, Wm = mask1.shape[-2], mask1.shape[-1]
    m1 = mask1[0, 0][(np.arange(mH) * Hm) // mH][:, (np.arange(mW) * Wm) // mW] >= 0.5
    m2 = mask2[0, 0][(np.arange(mh) * Hm) // mh][:, (np.arange(mw) * Wm) // mw] >= 0.5

    m1f = m1.reshape(-1)
    m2f = m2.reshape(-1)

    # group unmasked rows/cols first so masked-q chunks can use a short k loop
    qperm = np.argsort(m1f, kind="stable")       # False (unmasked) first
    kperm = np.argsort(m2f, kind="stable")
    n0 = int((~m1f).sum())
    m0 = int((~m2f).sum())
    use_sparse = n0 < N and m0 >= 128
    if not use_sparse:
        qperm = np.arange(N)
        kperm = np.arange(M)
        n0s, m0s = None, None
    else:
        n0s, m0s = n0, m0

    m1neg = np.where(m1f[qperm], np.float32(NEG), np.float32(0.0))
    m2col = m2f[kperm].astype(np.float32)
    xT = np.ascontiguousarray(x[0].T[:, qperm]).astype(bf16)
    ctxT = np.ascontiguousarray(context[0].T[:, kperm]).astype(bf16)

    def pack3(w):
        # [320, 64] -> [128, 192] (c-tiles of 128/128/64 side by side)
        p = np.zeros((128, 192), np.float32)
        p[:, 0:64] = w[0:128]
        p[:, 64:128] = w[128:256]
        p[0:64, 128:192] = w[256:320]
        return p

    def pack3kv(wk, wv):
        # [320, 64]x2 -> [128, 384]: per c-tile [cw, 128] = [k | v]
        p = np.zeros((128, 384), np.float32)
        for i, (c0, cw) in enumerate([(0, 128), (128, 128), (256, 64)]):
            p[0:cw, i * 128:i * 128 + 64] = wk[c0:c0 + cw]
            p[0:cw, i * 128 + 64:(i + 1) * 128] = wv[c0:c0 + cw]
        return p

    def wprep_pack(h):
        p = np.zeros((128, 640), np.float32)
        p[:, 0:192] = pack3(Wq[:, h * D:(h + 1) * D] * np.float32(SCALE))
        p[:, 192:576] = pack3kv(Wk[:, h * D:(h + 1) * D],
                                Wv[:, h * D:(h + 1) * D])
        p[0:64, 576:640] = np.eye(64, dtype=np.float32)
        return p.astype(bf16)

    in_maps = []
    for h in range(HEADS):
        in_maps.append({
            "xt": xT,
            "ctxt": ctxT,
            "wprep": wprep_pack(h),
            "wo": np.ascontiguousarray(Wo[h * D:(h + 1) * D, :]),
            "m1neg": m1neg.reshape(1, N),
            "m2col": m2col.reshape(1, M),
        })
    _last_in_maps = in_maps
    _last_key = (n0s, m0s)

    nc = _get_compiled(n0s, m0s)
    res = bass_utils.run_bass_kernel_spmd(nc, in_maps, list(range(HEADS)))
    outT = np.zeros((C, N), dtype=np.float32)
    for h in range(HEADS):
        r = res.results[h]
        outT += r["out"] / r["rsum"].reshape(1, N)
    out = outT.T + bo
    inv = np.empty(N, dtype=np.int64)
    inv[qperm] = np.arange(N)
    out = out[inv]
    return np.ascontiguousarray(out).reshape(1, N, C)
